# revision 54
# baseline (speedup 1.0000x reference)
"""Trainium2 Bass kernel for a 2-layer GAT + attention pooling (PyG-style).

Strategy (8 NeuronCores, SPMD):
  * Destination nodes are range-partitioned: core k owns dst nodes
    [k*N/8, (k+1)*N/8). Edges (self-loops appended) are sorted by dst and
    routed to the owning core (host-side index prep only).
  * conv1's dense part (h1 = x @ W1.T, per-node a_src) is computed
    replicated on every core (bf16) into a packed DRAM table:
    row(n) = [h1(n) | a_src(n)], row stride 640 bf16 (dma_gather needs
    256B-multiple rows).
  * Per-edge work: one dma_gather of table rows by src id, spread over
    4 SWDGE queues (single-queue gathers serialize call-by-call on real
    hw, ~25x slower for small rows). Within each chunk, edges are sorted
    by (src half, dst) and each gather call's table operand is sliced to
    the row prefix it needs, so conv1 gathers overlap the ext1 build.
  * The dst-side attention values are expanded per edge with two tiny
    matmuls through the <=32 distinct dsts per 128-edge tile (host
    ships compact selectors; per-tile mm1s write a block-diagonal small
    matrix, one whole-window mm2 expands 3 tiles at once), avoiding PE
    transposes and big PSUM->SBUF selector copies.
  * Weighted segment-sum rides the PE via the per-tile one-hot selector
    (built on-chip from shipped dloc values), accumulating numerator and
    denominator in PSUM per dst chunk of 128 nodes. Edge tiles are
    allocated per chunk (variable count) to minimise padding.
  * conv2 table rows are 128-col (h2 only; attention dots computed
    on-chip via a basis fold); the inter-conv AllGather is split into 3
    node-range collectives issued as chunks finish so they overlap the
    conv1 edge phase (per-collective overhead ~30us on hw dominates
    payload). The final head is folded per-core before a tiny [256,2]
    AllGather + on-chip sum (cheaper than AllReduce) of graph-pooling
    partials.
"""

import numpy as np
from contextlib import ExitStack

import concourse.bass as bass
import concourse.tile as tile
from concourse import bacc, mybir

F32 = mybir.dt.float32
F32R = mybir.dt.float32r
BF16 = mybir.dt.bfloat16
U8 = mybir.dt.uint8
I16 = mybir.dt.int16
AL = mybir.AluOpType
AF = mybir.ActivationFunctionType

TILE = 128
NEG = 0.2
KSL = 32  # selector slots per tile (max distinct dsts per 128-edge tile)
NSPLIT = 3  # AllGather splits (by chunk ranges)
SP_CHUNKS = [0, 9, 16, 20]  # split boundaries in chunks


class Cfg:
    def __init__(self, N=20000, NC=8, GRP=32, H1=8, C1=64, FIN=128, D2=128,
                 NGR=256, OUT=2):
        self.N, self.NC, self.GRP = N, NC, GRP
        self.H1, self.C1, self.FIN, self.D2 = H1, C1, FIN, D2
        self.D1 = H1 * C1
        self.NGR, self.OUT = NGR, OUT
        self.NLOC = N // NC
        self.CH = (self.NLOC + TILE - 1) // TILE
        self.EXT1 = 640                  # conv1 table row stride (bf16)
        self.ROW1 = self.D1 + H1        # written cols: h | a_src
        self.NT1 = (N + TILE - 1) // TILE
        chs = SP_CHUNKS
        assert chs[-1] == self.CH and len(chs) == NSPLIT + 1
        self.sp_ch = chs
        self.sp_rows = [min(self.NLOC, chs[s + 1] * TILE) -
                        min(self.NLOC, chs[s] * TILE) for s in range(NSPLIT)]
        self.sp_r0 = [min(self.NLOC, chs[s] * TILE) for s in range(NSPLIT)]
        self.sp_base = np.cumsum([0] + [r * NC for r in self.sp_rows]).tolist()


def build_program(cfg, cs, t_pad, g1rows=None, stage=99, dbg=False):
    """cs: per-chunk tile start offsets (len CH+1); cs[-1] extended to t_pad
    by folding trailing pad tiles into the last chunk. g1rows: per
    gather-call ext1 row prefix for sliced table dependencies."""
    c_ = cfg
    ng = t_pad // c_.GRP
    nt1 = c_.NT1

    nc = bacc.Bacc("TRN2", target_bir_lowering=False, debug=False,
                   num_devices=c_.NC, num_swdge_queues=4)

    def par(name, shape, dt=F32):
        return nc.declare_dram_parameter(name, shape, dt, isOutput=False)

    smw = c_.D1 + 4 * c_.D2 + TILE + c_.OUT + 4   # smalls width
    xT_p = par("x", [c_.FIN, nt1 * TILE], BF16)    # x transposed bf16 (host)
    xlocT_p = par("xloc", [c_.FIN, c_.CH * TILE], BF16)
    w1tb = par("w1tb", [c_.FIN, c_.D1], BF16)      # W1.T bf16
    wpa = par("wpa", [c_.D2, c_.D1])               # w2 (f32)
    wpb = par("wpb", [2 * c_.D1, c_.D2])           # w1 | w2t@B2
    apk = par("apk", [c_.D1, 2 * c_.H1 + 4])       # a1m | a2m | woutt
    gidx = par("gidx", [TILE, 2 * t_pad * 8], I16)  # gsrc1 | gsrc2
    dlp = par("dlp", [TILE, t_pad + TILE], BF16)   # dloc | iotaF
    selp = par("selp", [TILE, c_.CH * c_.NGR], BF16)  # b01 pooling sel
    nwin = -(-c_.GRP // 3)  # 3-tile selector windows per group
    selb = par("selb", [TILE, ng * nwin * TILE], BF16)  # b32 per window
    selt = par("selt", [TILE, ng * nwin * TILE], BF16)  # sel32T windows
    smalls = par("smalls", [TILE, smw])
    tok = par("tok", [TILE, 8])
    w2 = wpa
    w1 = wpb[0:c_.D1, :]
    w2t = wpb[c_.D1:2 * c_.D1, :]
    a1m = apk[:, 0:2 * c_.H1]
    a2m = apk[0:c_.D2, 2 * c_.H1:2 * c_.H1 + 2]
    woutt = apk[0:c_.D2, 2 * c_.H1 + 2:2 * c_.H1 + 4]
    gsrc1 = gidx[:, 0:t_pad * 8]
    gsrc2 = gidx[:, t_pad * 8:2 * t_pad * 8]
    o_ = [0]
    def sl(w):
        a = o_[0]; o_[0] += w
        return smalls[:, a:a + w]
    b1r = sl(c_.D1)
    b2r = sl(c_.D2)
    wattnr = sl(c_.D2)
    wmaskr = sl(c_.D2)
    as2r = sl(c_.D2)
    inv2 = sl(1)
    ident = sl(TILE)
    boutr = sl(c_.OUT)
    battn = sl(1)
    epsr = sl(1)
    bmask = sl(1)

    out_p = nc.declare_dram_parameter("out", [c_.NGR, c_.OUT], F32,
                                      isOutput=True)
    dbgh_p = nc.declare_dram_parameter(
        "dbgh", [TILE, c_.CH * c_.D1], BF16, isOutput=True) if dbg else None
    dbgo_p = nc.declare_dram_parameter(
        "dbgo", [TILE, c_.CH * c_.D2], F32, isOutput=True) if dbg else None
    tok_out = nc.declare_dram_parameter("tok_out", [TILE, 8], F32,
                                        isOutput=True)

    ext1 = nc.dram_tensor("ext1", [nt1 * TILE, c_.EXT1], BF16)
    ag_in = nc.dram_tensor("ag_in", [c_.NLOC, c_.D2], BF16)
    ext2 = nc.dram_tensor("ext2", [c_.N, c_.D2], BF16, addr_space="Shared")
    ar_in = nc.dram_tensor("ar_in", [c_.NGR, c_.OUT], F32)
    agred = nc.dram_tensor("agred", [c_.NC * c_.NGR, c_.OUT], F32,
                           addr_space="Shared")

    chunk_start = [int(cs[c]) for c in range(c_.CH)]
    chunk_end = [int(cs[c + 1]) for c in range(c_.CH)]
    chunk_end[c_.CH - 1] = t_pad
    chunk_of = np.zeros(t_pad, np.int32)
    for c in range(c_.CH):
        chunk_of[chunk_start[c]:chunk_end[c]] = c

    # a_dst-expand windows: 3 tiles share a 128-partition selector block
    # (32 slots each at bases 0/32/64). Per-tile mm1s write a
    # block-diagonal [slots, 3*nh] small matrix (each tile's rhs variant
    # zero-pads the other bands), so a single whole-window mm2 expands
    # all 3 tiles at once. Windows never cross group boundaries.
    win_by_group = []
    for g in range(ng):
        wins = []
        for wl in range(nwin):
            t0 = g * c_.GRP + wl * 3
            nt = min(3, (g + 1) * c_.GRP - t0)
            rr = [(i, int(chunk_of[t0 + i])) for i in range(nt)]
            wins.append((wl, t0, nt, rr))
        win_by_group.append(wins)

    d1c = c_.D1 // TILE   # f-chunks of D1
    g_half = c_.NGR // 2
    SB = 8                # sub-group tiles (dependency granularity)
    XB0 = 8               # x tiles per DMA

    with tile.TileContext(nc) as tc, ExitStack() as ctx:
        const = ctx.enter_context(tc.tile_pool(name="const", bufs=1))
        io = ctx.enter_context(tc.tile_pool(name="io", bufs=3))
        gat = ctx.enter_context(tc.tile_pool(name="gat", bufs=2))
        sm = ctx.enter_context(tc.tile_pool(name="sm", bufs=2))
        sel = ctx.enter_context(tc.tile_pool(name="sel", bufs=2))
        pp = ctx.enter_context(tc.tile_pool(name="pp", bufs=2, space="PSUM"))
        ppt = ctx.enter_context(tc.tile_pool(name="ppt", bufs=2, space="PSUM"))
        pc2 = ctx.enter_context(tc.tile_pool(name="pc2", bufs=1, space="PSUM"))

        _lc = [0]
        def load_const(p, shape, dt=F32):
            _lc[0] += 1
            t = const.tile(shape, dt, tag=f"c_{_lc[0]}")
            nc.sync.dma_start(t[:], p if isinstance(p, bass.AP) else p[:])
            return t

        ident_t = load_const(ident, [TILE, TILE])
        tok_t = const.tile([TILE, 8], F32, tag="tok")
        nc.sync.dma_start(tok_t[:], tok[:])
        nc.sync.dma_start(tok_out[:], tok_t[:])
        w1t_t = load_const(w1tb, [c_.FIN, c_.D1], BF16)
        woutt_t = load_const(woutt, [c_.D2, c_.OUT])
        b2r_t = load_const(b2r, [TILE, c_.D2])
        boutr_t = load_const(boutr, [TILE, c_.OUT])
        wattn_t = load_const(wattnr, [TILE, c_.D2])
        wmask_t = load_const(wmaskr, [TILE, c_.D2])
        battn_t = load_const(battn, [TILE, 1])
        eps_t = load_const(epsr, [TILE, 1])
        bmask_t = load_const(bmask, [TILE, 1])
        gsrc1_t = load_const(gsrc1, [TILE, t_pad * 8], I16)
        gsrc2_t = load_const(gsrc2, [TILE, t_pad * 8], I16)
        a2m_t = load_const(a2m, [c_.D2, 2])
        w2_t = load_const(w2, [c_.D2, c_.D1])
        dloc_t = load_const(dlp[:, 0:t_pad], [TILE, t_pad], BF16)
        iotaF_t = load_const(dlp[:, t_pad:t_pad + TILE], [TILE, TILE], BF16)
        b1r_f = load_const(b1r, [TILE, c_.D1])
        as2mod_t = load_const(as2r, [TILE, c_.D2])
        inv2_t = load_const(inv2, [TILE, 1])

        w2t_t = const.tile([TILE, d1c, c_.D2], F32)
        a1m_t = const.tile([TILE, d1c, 2 * c_.H1], F32)
        w1_t = const.tile([TILE, d1c, c_.FIN], F32)
        for dc in range(d1c):
            nc.sync.dma_start(w2t_t[:, dc, :], w2t[dc * TILE:(dc + 1) * TILE, :])
            nc.sync.dma_start(a1m_t[:, dc, :], a1m[dc * TILE:(dc + 1) * TILE, :])
            nc.sync.dma_start(w1_t[:, dc, :], w1[dc * TILE:(dc + 1) * TILE, :])

        # bf16 copies of conv2 weights / conv1 bias
        w2t_bf = const.tile([TILE, d1c, c_.D2], BF16)
        for dc in range(d1c):
            nc.vector.tensor_copy(w2t_bf[:, dc, :], w2t_t[:, dc, :])
        b1r_t = const.tile([TILE, c_.D1], BF16)
        nc.vector.tensor_copy(b1r_t[:], b1r_f[:])
        ident_bf = const.tile([TILE, TILE], BF16)
        nc.vector.tensor_copy(ident_bf[:], ident_t[:])

        # A1eff = W1.T @ A1m  [FIN, 2*H1];  A2eff = W2.T @ A2m  [D1, 2]
        a1eff_ps = ppt.tile([c_.FIN, 2 * c_.H1], F32, tag="tps")
        for dc in range(d1c):
            nc.tensor.matmul(a1eff_ps[:], w1_t[:, dc, :], a1m_t[:, dc, :],
                             start=(dc == 0), stop=(dc == d1c - 1))
        a1eff = const.tile([c_.FIN, 2 * c_.H1], BF16)
        nc.vector.tensor_copy(a1eff[:], a1eff_ps[:])

        a2eff = const.tile([TILE, d1c, 2], BF16)
        for fc in range(d1c):
            a2eff_ps = ppt.tile([TILE, 2], F32, tag="tps")
            nc.tensor.matmul(a2eff_ps[:], w2_t[:, fc * TILE:(fc + 1) * TILE],
                             a2m_t[:], start=True, stop=True)
            nc.vector.tensor_copy(a2eff[:, fc, :], a2eff_ps[:])

        # local a_dst tables, 3 banded variants each (band a holds the
        # values, other bands zero) for block-diagonal window mm1s:
        # conv1 [TILE, CH, 3*H1] x3, conv2 [TILE, CH, 3] x3
        adloc1 = [const.tile([TILE, c_.CH, 3 * c_.H1], BF16,
                             name=f"adloc1_{a}", tag=f"adloc1_{a}")
                  for a in range(3)]
        adloc2 = [const.tile([TILE, c_.CH, 3], BF16, name=f"adloc2_{a}",
                             tag=f"adloc2_{a}") for a in range(3)]
        for a in range(3):
            nc.vector.memset(adloc1[a][:], 0.0)
            nc.vector.memset(adloc2[a][:], 0.0)
        for cb in range((c_.CH + XB0 - 1) // XB0):
            nb = min(XB0, c_.CH - cb * XB0)
            xl = io.tile([c_.FIN, XB0 * TILE], BF16, tag="xt")
            nc.sync.dma_start(xl[:, 0:nb * TILE],
                              xlocT_p[:, cb * XB0 * TILE:
                                      (cb * XB0 + nb) * TILE])
            for i in range(nb):
                c = cb * XB0 + i
                ad_ps = ppt.tile([TILE, c_.H1], F32, tag="tps")
                nc.tensor.matmul(ad_ps[:], xl[:, i * TILE:(i + 1) * TILE],
                                 a1eff[:, c_.H1:2 * c_.H1],
                                 start=True, stop=True)
                for a in range(3):
                    nc.vector.tensor_copy(
                        adloc1[a][:, c, a * c_.H1:(a + 1) * c_.H1], ad_ps[:])

        # =========== conv1 phase 1: replicated ext1 table ===========
        xt4 = None
        stg = None
        for t in range(nt1 if stage >= 1 else 0):
            if t % XB0 == 0:
                nb = min(XB0, nt1 - t)
                xt4 = io.tile([c_.FIN, XB0 * TILE], BF16, tag="xt")
                nc.sync.dma_start(xt4[:, 0:nb * TILE],
                                  xT_p[:, t * TILE:(t + nb) * TILE])
            xt = xt4[:, (t % XB0) * TILE:(t % XB0 + 1) * TILE]
            h_ps = pp.tile([TILE, c_.ROW1], F32, tag="pnum")
            nc.tensor.matmul(h_ps[:, 0:c_.D1], xt, w1t_t[:],
                             start=True, stop=True)
            nc.tensor.matmul(h_ps[:, c_.D1:c_.ROW1], xt, a1eff[:, 0:c_.H1],
                             start=True, stop=True)
            if t % 4 == 0:
                stg = io.tile([TILE, 4, c_.ROW1], BF16, tag="stg")
            sv = stg[:, t % 4, :]
            if t % 2 == 0:
                nc.scalar.copy(sv[:], h_ps[:])
            else:
                nc.vector.tensor_copy(sv[:], h_ps[:])
            if t % 4 == 3 or t == nt1 - 1:
                t0w = t - (t % 4)
                nj = t - t0w + 1
                dst = ext1[t0w * TILE:(t0w + nj) * TILE, 0:c_.ROW1].rearrange(
                    "(j p) e -> p j e", p=TILE)
                nc.sync.dma_start(dst, stg[:, 0:nj, :])

        h_own = const.tile([TILE, c_.CH, c_.D1], BF16)
        o2_own = const.tile([TILE, c_.CH, c_.D2], F32)

        # =========== conv2 phase 1 for a chunk + AG split ===========
        def conv2_p1(c):
            h2_ps = pc2.tile([TILE, c_.D2], F32, tag="c2h")
            a2_ps = pc2.tile([TILE, 2], F32, tag="c2a")
            for fc in range(d1c):
                hT_ps = ppt.tile([TILE, TILE], BF16, tag="tps")
                nc.tensor.transpose(
                    hT_ps[:], h_own[:, c, fc * TILE:(fc + 1) * TILE],
                    ident_bf[:])
                hT = io.tile([TILE, TILE], BF16, tag="hT")
                nc.vector.tensor_copy(hT[:], hT_ps[:])
                nc.tensor.matmul(h2_ps[:], hT[:], w2t_bf[:, fc, :],
                                 start=(fc == 0), stop=(fc == d1c - 1))
                nc.tensor.matmul(a2_ps[:], hT[:],
                                 a2eff[:, fc, :],
                                 start=(fc == 0), stop=(fc == d1c - 1))
            stg2 = io.tile([TILE, c_.D2], BF16, tag="stg2")
            nc.vector.tensor_copy(stg2[:], h2_ps[:])
            for a in range(3):
                nc.vector.tensor_copy(adloc2[a][:, c, a:a + 1],
                                      a2_ps[:, 1:2])
            rows = min(TILE, c_.NLOC - c * TILE)
            nc.sync.dma_start(ag_in[c * TILE:c * TILE + rows, :],
                              stg2[0:rows, :])

        def ag_split(s):
            r0, rws = c_.sp_r0[s], c_.sp_rows[s]
            nc.gpsimd.collective_compute(
                "AllGather", AL.bypass, replica_groups=[list(range(c_.NC))],
                ins=[ag_in[r0:r0 + rws, :]],
                outs=[ext2[c_.sp_base[s]:c_.sp_base[s + 1], :]])

        # =========== shared edge-aggregation pipeline ===========
        def conv_phase2(conv, do_split):
            # conv==1: ext1 rows (640 stride), nh=H1, dfeat=D1
            # conv==2: ext2 rows (128 stride), nh=1, dfeat=D2
            if conv == 1:
                nh, dfeat, adloc, bias_t, dst_sb = \
                    c_.H1, c_.D1, adloc1, b1r_t, h_own
                gs_t = gsrc1_t
            else:
                nh, dfeat, adloc, bias_t, dst_sb = \
                    1, c_.D2, adloc2, b2r_t, None
                gs_t = gsrc2_t
            psn = None
            next_split = [0]
            cph = dfeat // nh

            GH = c_.GRP // 4  # quarter-group gather granularity
            def issue_gather(g):
                if conv == 1:
                    tab, cols = ext1, c_.EXT1
                    extg = gat.tile([TILE, c_.GRP, c_.EXT1], BF16, tag="extg")
                else:
                    tab, cols = ext2, c_.D2
                    extg = gat.tile([TILE, c_.GRP, c_.D2], BF16, tag="extg")
                for hh in range(4):
                    if conv == 1 and g1rows is not None:
                        rows = int(g1rows[g * 4 + hh])
                    else:
                        rows = tab.shape[0]
                    nc.gpsimd.dma_gather(
                        extg[:, hh * GH:(hh + 1) * GH, :], tab[0:rows, 0:cols],
                        gs_t[:, (g * c_.GRP + hh * GH) * 8:
                             (g * c_.GRP + (hh + 1) * GH) * 8],
                        GH * TILE, GH * TILE, cols,
                        elem_step=cols, single_packet=(conv == 1),
                        queue_num=hh)
                return extg

            nxt = issue_gather(0)
            for g in range(ng):
                extg = nxt
                if g + 1 < ng:
                    nxt = issue_gather(g + 1)

                adt = BF16 if conv == 1 else F32
                s01g = sel.tile([TILE, c_.GRP, TILE], BF16, tag="s01g")
                b8g = sel.tile([TILE, nwin, TILE], BF16, tag="b8g")
                sel8g = sel.tile([TILE, nwin, TILE], BF16, tag="sel8g")
                nc.sync.dma_start(
                    b8g[:], selb[:, g * nwin * TILE:(g + 1) * nwin * TILE]
                    .rearrange("p (b e) -> p b e", e=TILE))
                nc.sync.dma_start(
                    sel8g[:], selt[:, g * nwin * TILE:(g + 1) * nwin * TILE]
                    .rearrange("p (b e) -> p b e", e=TILE))
                advg = sm.tile([TILE, c_.GRP, nh], adt, tag="advg")
                pg = sm.tile([TILE, c_.GRP, nh], BF16, tag="pg")
                if conv == 2:
                    sa_t = sm.tile([TILE, c_.GRP, 1], F32, tag="sa2")
                dld = sm.tile([TILE, c_.GRP, 2], BF16, tag="dld")
                nc.scalar.copy(dld[:], dloc_t[:, g * c_.GRP:(g + 1) * c_.GRP]
                               .unsqueeze(2).broadcast_to([TILE, c_.GRP, 2]))
                # --- selector build via iota compare (batched)
                EQB = 4
                for b in range(c_.GRP // EQB):
                    bo = b * EQB
                    dl = dld[:, bo:bo + EQB, :].unsqueeze(2).broadcast_to(
                        [TILE, EQB, TILE // 2, 2])
                    io_f = iotaF_t[:].rearrange("p (a b) -> p a b", b=2) \
                        .unsqueeze(1).broadcast_to([TILE, EQB, TILE // 2, 2])
                    s01v = s01g[:, bo:bo + EQB, :].rearrange(
                        "p g (a b) -> p g a b", b=2)
                    nc.vector.tensor_tensor(s01v, io_f, dl, AL.is_equal)

                # --- a_dst expand: per-tile mm1s fill a block-diagonal
                # [slots, 3*nh] small matrix; one whole-window mm2 expands
                # all 3 tiles' a_dst values at once.
                for (wl, t0, nt, rr) in win_by_group[g]:
                    tl0 = t0 % c_.GRP
                    small_ps = ppt.tile([TILE, 3 * nh], F32, tag="tps")
                    for (a, c) in rr:
                        nc.tensor.matmul(
                            small_ps[a * KSL:(a + 1) * KSL, 0:nt * nh],
                            b8g[:, wl, a * KSL:(a + 1) * KSL],
                            adloc[a][:, c, 0:nt * nh],
                            start=True, stop=True)
                    small_sb = sm.tile([TILE, 3 * nh], BF16, tag="smsb")
                    nc.scalar.copy(small_sb[0:nt * KSL, 0:nt * nh],
                                   small_ps[0:nt * KSL, 0:nt * nh])
                    adv_ps = ppt.tile([TILE, 3, nh], F32, tag="tps")
                    nc.tensor.matmul(
                        adv_ps[:, 0:nt, :].rearrange("p a h -> p (a h)"),
                        sel8g[0:nt * KSL, wl, :],
                        small_sb[0:nt * KSL, 0:nt * nh],
                        start=True, stop=True)
                    nc.scalar.copy(advg[:, tl0:tl0 + nt, :],
                                   adv_ps[:, 0:nt, :])

                for sub in range(c_.GRP // SB):
                    sl_ = slice(sub * SB, (sub + 1) * SB)
                    if conv == 1:
                        sa = extg[:, sl_, c_.D1:c_.D1 + nh]
                        nc.vector.tensor_tensor(sa, sa, advg[:, sl_, :],
                                                AL.add)
                    else:
                        # a_src2 rides gathered col 0 (basis-folded on host)
                        sa = sa_t[:, sl_, :]
                        nc.vector.tensor_tensor(sa, extg[:, sl_, 0:1],
                                                advg[:, sl_, :], AL.add)
                    tmp = sm.tile([TILE, SB, nh], adt, tag="tmp")
                    nc.scalar.mul(tmp[:], sa, NEG)
                    nc.vector.tensor_tensor(sa, sa, tmp[:], AL.max)
                    nc.scalar.activation(pg[:, sl_, :], sa, AF.Exp)
                    # duplicate alpha x2 so the multiply's operands are all
                    # packed in the last dim (DVE 2x mode)
                    pexp = sm.tile([TILE, SB, nh, 2], BF16, tag="pexp")
                    nc.scalar.copy(pexp[:], pg[:, sl_, :].unsqueeze(
                        3).broadcast_to([TILE, SB, nh, 2]))

                    # --- alpha-weight the gathered rows (packed-pair view)
                    ev = extg[:, sl_, 0:dfeat].rearrange(
                        "p g (h a b) -> p g h a b", h=nh, b=2)
                    pb = pexp[:].unsqueeze(3).broadcast_to(
                        [TILE, SB, nh, cph // 2, 2])
                    nc.vector.tensor_tensor(ev, ev, pb, AL.mult)

                    # --- segment-sum via selector matmuls
                    for i in range(SB):
                        tl = sub * SB + i
                        t = g * c_.GRP + tl
                        c = int(chunk_of[t])
                        first = t == chunk_start[c]
                        last = t == chunk_end[c] - 1
                        if first:
                            psn = pp.tile([TILE, 512 + nh], F32, tag="pnum")
                        lhs = s01g[:, tl, :]
                        nc.tensor.matmul(psn[:, 0:dfeat], lhs,
                                         extg[:, tl, 0:dfeat],
                                         start=first, stop=last)
                        nc.tensor.matmul(psn[:, 512:512 + nh], lhs,
                                         pg[:, tl, :],
                                         start=first, stop=last)
                        if not last:
                            continue
                        den = sm.tile([TILE, nh], F32, tag="den")
                        nc.vector.tensor_scalar_add(den[:],
                                                    psn[:, 512:512 + nh],
                                                    1e-16)
                        denr = sm.tile([TILE, nh], F32, tag="denr")
                        nc.vector.reciprocal(denr[:], den[:])
                        if conv == 1:
                            for h in range(nh):
                                nc.scalar.activation(
                                    dst_sb[:, c, h * cph:(h + 1) * cph],
                                    psn[:, h * cph:(h + 1) * cph], AF.Copy,
                                    scale=denr[:, h:h + 1])
                            nc.vector.tensor_tensor(
                                dst_sb[:, c, :], dst_sb[:, c, :], bias_t[:],
                                AL.add)
                            nc.vector.tensor_scalar_max(
                                dst_sb[:, c, :], dst_sb[:, c, :], 0.0)
                            if do_split:
                                # conv2 dense part per chunk as it finishes
                                # (spreads PE load); fire the AllGather as
                                # soon as its last chunk is done
                                conv2_p1(c)
                                if next_split[0] < NSPLIT and \
                                        c + 1 == c_.sp_ch[next_split[0] + 1]:
                                    ag_split(next_split[0])
                                    next_split[0] += 1
                        else:
                            o2 = o2_own[:, c, :]
                            nc.scalar.activation(o2, psn[:, 0:dfeat], AF.Copy,
                                                 scale=denr[:])
                            # undo the a_src2 basis fold on column 0:
                            # h2[0] = (M0 - sum_j>0 as2[j]*M[j]) / as2[0]
                            scr2 = sm.tile([TILE, c_.D2], F32, tag="scr2")
                            acc2 = sm.tile([TILE, 1], F32, tag="acc2")
                            nc.vector.tensor_tensor(scr2[:], o2, as2mod_t[:],
                                                    AL.mult)
                            nc.vector.tensor_reduce(
                                acc2[:], scr2[:], mybir.AxisListType.X, AL.add)
                            t1 = sm.tile([TILE, 1], F32, tag="t1c")
                            nc.vector.tensor_tensor(t1[:], o2_own[:, c, 0:1],
                                                    acc2[:], AL.subtract)
                            nc.scalar.activation(o2_own[:, c, 0:1], t1[:],
                                                 AF.Copy, scale=inv2_t[:])
                            nc.vector.tensor_tensor(o2, o2, bias_t[:], AL.add)
                            nc.vector.tensor_scalar_max(o2, o2, 0.0)

        if stage >= 2:
            conv_phase2(1, do_split=(stage >= 3))
        else:
            nc.vector.memset(h_own[:], 0.0)

        if stage == 2:
            for cc in range(c_.CH):
                conv2_p1(cc)

        # =========== conv2 phase 2 ===========
        if stage >= 4:
            conv_phase2(2, do_split=False)
        else:
            nc.vector.memset(o2_own[:], 0.0)

        if dbg:
            nc.sync.dma_start(dbgh_p[:, :],
                              h_own[:].rearrange("p c d -> p (c d)"))
            nc.sync.dma_start(dbgo_p[:, :],
                              o2_own[:].rearrange("p c d -> p (c d)"))

        # =========== pooling partials + folded head + tiny AllReduce =====
        ps_pa = pp.tile([TILE, c_.D2], F32, tag="pnum")
        ps_pb = pc2.tile([TILE, c_.D2], F32, tag="c2h")
        nch_pool = c_.CH if stage >= 5 else 1
        for c in range(nch_pool):
            h = o2_own[:, c, :]
            ta = sm.tile([TILE, c_.D2], F32, tag="ta")
            nc.vector.tensor_tensor(ta[:], h, wattn_t[:], AL.mult)
            sa = sm.tile([TILE, 1], F32, tag="sa")
            nc.vector.tensor_reduce(sa[:], ta[:], mybir.AxisListType.X, AL.add)
            nc.scalar.activation(sa[:], sa[:], AF.Identity, bias=battn_t[:])
            tm = sm.tile([TILE, c_.D2], F32, tag="ta")
            nc.vector.tensor_tensor(tm[:], h, wmask_t[:], AL.mult)
            sb = sm.tile([TILE, 1], F32, tag="sb")
            nc.vector.tensor_reduce(sb[:], tm[:], mybir.AxisListType.X, AL.add)
            nc.scalar.activation(sb[:], sb[:], AF.Sigmoid, bias=bmask_t[:])
            fac = sm.tile([TILE, 1], F32, tag="fac")
            nc.vector.tensor_tensor(fac[:], sa[:], sb[:], AL.mult)
            wn = sm.tile([TILE, c_.D2], BF16, tag="wn")
            nc.scalar.activation(wn[:], h, AF.Copy, scale=fac[:])
            b01g = sm.tile([TILE, c_.NGR], BF16, tag="b01g")
            nc.sync.dma_start(b01g[:], selp[:, c * c_.NGR:(c + 1) * c_.NGR])
            nc.tensor.matmul(ps_pa[:], b01g[:, 0:g_half], wn[:],
                             start=(c == 0), stop=(c == nch_pool - 1))
            nc.tensor.matmul(ps_pb[:], b01g[:, g_half:c_.NGR], wn[:],
                             start=(c == 0), stop=(c == nch_pool - 1))
        # per-core folded head: partial_out = pooled_partial @ W_out.T
        for half, ps in ((0, ps_pa), (1, ps_pb)):
            pl = io.tile([g_half, c_.D2], F32, tag="pl")
            nc.vector.tensor_copy(pl[:], ps[0:g_half, :])
            pT_ps = ppt.tile([c_.D2, g_half], F32, tag="tps")
            nc.tensor.transpose(pT_ps[:], pl[:], ident_t[0:g_half, 0:g_half])
            pT = io.tile([c_.D2, g_half], F32, tag="xTp")
            nc.vector.tensor_copy(pT[:], pT_ps[:])
            o_ps = ppt.tile([g_half, c_.OUT], F32, tag="tps")
            nc.tensor.matmul(o_ps[:], pT[:], woutt_t[:], start=True, stop=True)
            ot = io.tile([g_half, c_.OUT], F32, tag="ot")
            nc.vector.tensor_copy(ot[:], o_ps[:])
            nc.sync.dma_start(ar_in[half * g_half:(half + 1) * g_half, :],
                              ot[:])

        # AllGather the per-core [256,2] head partials, sum on-chip
        # (cheaper than an AllReduce on real hw), add b_out once.
        of_all = io.tile([g_half, 2 * c_.NC, c_.OUT], F32, tag="of")
        if stage >= 1:
            nc.gpsimd.collective_compute(
                "AllGather", AL.bypass, replica_groups=[list(range(c_.NC))],
                ins=[ar_in[:]], outs=[agred[:]])
            nc.sync.dma_start(
                of_all[:], agred[:].rearrange("(b p) e -> p b e", p=g_half))
        else:
            nc.vector.memset(of_all[:], 0.0)
            nc.sync.dma_start(
                of_all[:, 0:2, :],
                ar_in[:].rearrange("(b p) e -> p b e", p=g_half))
        acc = io.tile([g_half, 2, c_.OUT], F32, tag="ofacc")
        nc.vector.tensor_tensor(acc[:], of_all[:, 0:2, :],
                                of_all[:, 2:4, :], AL.add)
        for k in range(2, c_.NC):
            nc.vector.tensor_tensor(acc[:], acc[:],
                                    of_all[:, 2 * k:2 * k + 2, :], AL.add)
        nc.vector.tensor_tensor(
            acc[:], acc[:],
            boutr_t[0:g_half, :].unsqueeze(1).broadcast_to(
                [g_half, 2, c_.OUT]), AL.add)
        nc.sync.dma_start(out_p[:].rearrange("(j p) e -> p j e", p=g_half),
                          acc[:])

    nc.compile()
    return nc


def host_prep(inputs, cfg):
    c_ = cfg
    ei = np.asarray(inputs["edge_index"], np.int64)
    batch = np.asarray(inputs["batch"], np.int64)

    loops = np.arange(c_.N, dtype=np.int64)
    src = np.concatenate([ei[0], loops])
    dst = np.concatenate([ei[1], loops])
    order = np.argsort(dst, kind="stable")
    src_s = src[order]
    dst_s = dst[order]

    # within each (core, chunk): order edges by (src half, dst) so early
    # tiles only reference the first half of the ext1 table -> conv1
    # gathers can start while ext1 is still being written
    lo_hi = []
    tiles_c = np.zeros(c_.CH, np.int64)
    for k in range(c_.NC):
        row = []
        for c in range(c_.CH):
            d0 = k * c_.NLOC + c * TILE
            d1 = min(k * c_.NLOC + c_.NLOC, d0 + TILE)
            lo = int(np.searchsorted(dst_s, d0))
            hi = int(np.searchsorted(dst_s, d1))
            row.append((lo, hi, d0))
            tiles_c[c] = max(tiles_c[c], -(-(hi - lo) // TILE))
        lo_hi.append(row)
    cs = np.concatenate([[0], np.cumsum(tiles_c)])
    t_pad = -(-int(cs[-1]) // c_.GRP) * c_.GRP

    # ext2 (split-permuted) row index for a global node id
    sp_r0 = np.asarray(c_.sp_r0 + [c_.NLOC], np.int64)
    sp_base = np.asarray(c_.sp_base, np.int64)
    sp_rows = np.asarray(c_.sp_rows, np.int64)

    def ext2_row(n):
        core = n // c_.NLOC
        pos = n % c_.NLOC
        s = np.searchsorted(sp_r0, pos, side="right") - 1
        return sp_base[s] + core * sp_rows[s] + (pos - sp_r0[s])

    def wrap_idx(a):
        w = a.reshape(-1, 16).T.astype(np.int16)
        return np.tile(w, (8, 1)).copy()

    import ml_dtypes
    per_core = []
    for k in range(c_.NC):
        gi_src = np.zeros(t_pad * TILE, np.int64)
        dloc = np.full(t_pad * TILE, -1, np.int64)
        for c in range(c_.CH):
            lo, hi, d0 = lo_hi[k][c]
            cnt = hi - lo
            j = int(cs[c]) * TILE + np.arange(cnt)
            ss = src_s[lo:hi]
            dd = dst_s[lo:hi] - d0
            o2 = np.lexsort((dd, ss * 2 // c_.N))
            gi_src[j] = ss[o2]
            dloc[j] = dd[o2]
        gi_src2 = ext2_row(gi_src)
        # dloc in [TILE, t_pad] layout: col t holds tile t's 128 slots
        dl2 = dloc.reshape(t_pad, TILE)
        dl = dl2.T.astype(ml_dtypes.bfloat16)
        # compact per-tile selectors: b16 [dst->slot], sel32T [slot->edge];
        # 3 tiles share a 128-partition window (32 slots at bases 0/32/64)
        GRP = c_.GRP
        nwin = -(-GRP // 3)
        ng = t_pad // GRP
        b16 = np.zeros((TILE, ng * nwin * TILE), ml_dtypes.bfloat16)
        selt = np.zeros((TILE, ng * nwin * TILE), ml_dtypes.bfloat16)
        for t in range(t_pad):
            dd = dl2[t]
            m = dd >= 0
            if not m.any():
                continue
            u, inv = np.unique(dd[m], return_inverse=True)
            assert len(u) <= KSL, f"tile {t}: {len(u)} distinct dsts"
            g, tl = t // GRP, t % GRP
            blk = g * nwin + tl // 3
            a = tl % 3
            b16[u, blk * TILE + a * KSL + np.arange(len(u))] = 1
            ee = np.nonzero(m)[0]
            selt[a * KSL + inv, blk * TILE + ee] = 1
        b01 = np.zeros((TILE, c_.CH * c_.NGR), ml_dtypes.bfloat16)
        ii = np.arange(c_.NLOC)
        b01[ii % TILE, (ii // TILE) * c_.NGR + batch[k * c_.NLOC + ii]] = 1
        per_core.append({"gsrc1": wrap_idx(gi_src), "gsrc2": wrap_idx(gi_src2),
                         "dloc": dl, "b01": b01, "b16": b16, "selt": selt,
                         "_gi": gi_src})
    # per-gather-call ext1 row prefix (max over cores, 128-rounded): the
    # gather's table AP is sliced to this so it only depends on the ext1
    # writes it actually needs
    GH = 8
    ncall = t_pad // GH
    g1rows = np.zeros(ncall, np.int64)
    for pc in per_core:
        gi = pc.pop("_gi").reshape(ncall, GH * TILE)
        g1rows = np.maximum(g1rows, gi.max(axis=1) + 1)
    g1rows = np.minimum(-(-g1rows // TILE) * TILE, c_.NT1 * TILE)
    return cs, t_pad, per_core, g1rows


def make_in_maps(inputs, cfg, per_core, t_pad):
    import ml_dtypes
    c_ = cfg
    x = np.asarray(inputs["x"], np.float32)
    nt1 = c_.NT1
    x_pad = np.zeros((nt1 * TILE, c_.FIN), np.float32)
    x_pad[:c_.N] = x
    xT = np.ascontiguousarray(x_pad.T).astype(ml_dtypes.bfloat16)

    W1 = np.asarray(inputs["W1"], np.float32)
    as1 = np.asarray(inputs["att_src1"], np.float32)
    ad1 = np.asarray(inputs["att_dst1"], np.float32)
    W2 = np.asarray(inputs["W2"], np.float32)
    as2 = np.asarray(inputs["att_src2"], np.float32)
    ad2 = np.asarray(inputs["att_dst2"], np.float32)
    a1m = np.zeros((c_.D1, 2 * c_.H1), np.float32)
    for h in range(c_.H1):
        a1m[h * c_.C1:(h + 1) * c_.C1, h] = as1[h]
        a1m[h * c_.C1:(h + 1) * c_.C1, c_.H1 + h] = ad1[h]

    # permute conv2 feature space so argmax|att_src2| is feature 0, then
    # fold the a_src2 functional into that column of the conv2 weights
    # (M = h2p @ B2). The edge phase reads a_src2 as gathered col 0 and the
    # chunk epilogue inverts the fold.
    k2 = int(np.argmax(np.abs(as2[0])))
    perm2 = np.concatenate([[k2], np.delete(np.arange(c_.D2), k2)])
    W2p = W2[perm2, :]
    as2p = as2[0][perm2].astype(np.float32)
    ad2p = ad2[0][perm2].astype(np.float32)
    B2 = np.eye(c_.D2, dtype=np.float32)
    B2[:, 0] = as2p
    a2m = np.stack([as2p, ad2p], axis=1).astype(np.float32)
    as2mod = as2p.copy()
    as2mod[0] = 0.0

    rep = lambda v, w: np.tile(
        np.asarray(v, np.float32).reshape(1, w), (TILE, 1))
    w1tb = np.ascontiguousarray(W1.T).astype(ml_dtypes.bfloat16)
    wpa = W2p.copy()
    wpb = np.concatenate([W1, np.ascontiguousarray(W2p.T @ B2)], axis=0)
    apk = np.zeros((c_.D1, 2 * c_.H1 + 4), np.float32)
    apk[:, 0:2 * c_.H1] = a1m
    apk[0:c_.D2, 2 * c_.H1:2 * c_.H1 + 2] = a2m
    apk[0:c_.D2, 2 * c_.H1 + 2:2 * c_.H1 + 4] = np.ascontiguousarray(
        np.asarray(inputs["W_out"], np.float32)[:, perm2].T)
    smalls = np.concatenate([
        rep(np.asarray(inputs["b1"], np.float32), c_.D1),
        rep(np.asarray(inputs["b2"], np.float32)[perm2], c_.D2),
        rep(np.asarray(inputs["w_attn"], np.float32)[perm2, 0], c_.D2),
        rep(np.asarray(inputs["w_mask"], np.float32)[perm2, 0], c_.D2),
        rep(as2mod, c_.D2),
        np.full((TILE, 1), 1.0 / as2p[0], np.float32),
        np.eye(TILE, dtype=np.float32),
        rep(inputs["b_out"], c_.OUT),
        rep(inputs["b_attn"], 1),
        np.full((TILE, 1), 1e-16, np.float32),
        rep(inputs["b_mask"], 1),
    ], axis=1)
    iotaF = np.tile(np.arange(TILE, dtype=np.float32).reshape(1, TILE),
                    (TILE, 1)).astype(ml_dtypes.bfloat16)
    base = {
        "x": xT, "w1tb": w1tb, "wpa": wpa, "wpb": wpb, "apk": apk,
        "smalls": smalls, "tok": np.zeros((TILE, 8), np.float32),
    }
    in_maps = []
    for k in range(c_.NC):
        m = dict(base)
        pc = per_core[k]
        m["gidx"] = np.concatenate([pc["gsrc1"], pc["gsrc2"]], axis=1)
        m["dlp"] = np.concatenate([pc["dloc"], iotaF], axis=1)
        m["selp"] = pc["b01"]
        m["selb"] = pc["b16"]
        m["selt"] = pc["selt"]
        xloc = np.zeros((c_.FIN, c_.CH * TILE), ml_dtypes.bfloat16)
        xloc[:, 0:c_.NLOC] = xT[:, k * c_.NLOC:(k + 1) * c_.NLOC]
        m["xloc"] = xloc
        in_maps.append(m)
    return in_maps


_CACHE = {}


def run(inputs, cfg):
    from concourse.bass_utils import run_bass_kernel_spmd
    cs, t_pad, per_core, g1rows = host_prep(inputs, cfg)
    key = (cfg.N, t_pad, tuple(cs), tuple(g1rows))
    if key not in _CACHE:
        _CACHE[key] = build_program(cfg, cs, t_pad, g1rows)
    nc = _CACHE[key]
    in_maps = make_in_maps(inputs, cfg, per_core, t_pad)
    res = run_bass_kernel_spmd(nc, in_maps, list(range(cfg.NC)), trace=False)
    return np.asarray(res.results[0]["out"], np.float32)


def kernel(**inputs):
    return run(inputs, Cfg())


def _exec_maker(nc, in_maps, n_cores):
    """Build a jitted executor (structure identical to bass2jax's _body) and
    device-resident inputs. Returns (f, dev_args)."""
    import jax
    from jax.sharding import Mesh, PartitionSpec, NamedSharding
    from jax.experimental.shard_map import shard_map
    from concourse import mybir as mb
    from concourse.bass2jax import _bass_exec_p, partition_id_tensor, \
        install_neuronx_cc_hook

    install_neuronx_cc_hook()
    partition_name = (nc.partition_id_tensor.name
                      if nc.partition_id_tensor else None)
    in_names, out_names, out_avals, zero_outs = [], [], [], []
    for alloc in nc.m.functions[0].allocations:
        if not isinstance(alloc, mb.MemoryLocationSet):
            continue
        name = alloc.memorylocations[0].name
        if alloc.kind == "ExternalInput":
            if name != partition_name:
                in_names.append(name)
        elif alloc.kind == "ExternalOutput":
            out_names.append(name)
            shape = tuple(alloc.tensor_shape)
            dtype = mb.dt.np(alloc.dtype)
            out_avals.append(jax.core.ShapedArray(shape, dtype))
            zero_outs.append(np.zeros(shape, dtype))
    n_params = len(in_names)
    all_in = in_names + out_names
    if partition_name is not None:
        all_in = all_in + [partition_name]

    def _body(*args):
        ops = list(args)
        if partition_name is not None:
            ops.append(partition_id_tensor())
        outs = _bass_exec_p.bind(
            *ops, out_avals=tuple(out_avals), in_names=tuple(all_in),
            out_names=tuple(out_names), lowering_input_output_aliases=(),
            sim_require_finite=True, sim_require_nnan=True, nc=nc)
        return tuple(outs)

    devices = jax.devices()[:n_cores]
    mesh = Mesh(np.asarray(devices), ("core",))
    nin = n_params + len(zero_outs)
    f = jax.jit(shard_map(
        _body, mesh=mesh, in_specs=(PartitionSpec("core"),) * nin,
        out_specs=(PartitionSpec("core"),) * len(out_names),
        check_rep=False), keep_unused=True)
    per_core = [[np.asarray(in_maps[c][n]) for n in in_names] + zero_outs
                for c in range(n_cores)]
    concat_in = [np.concatenate([per_core[c][i] for c in range(n_cores)],
                                axis=0) for i in range(nin)]
    sh = NamedSharding(mesh, PartitionSpec("core"))
    dev_args = [jax.device_put(a, sh) for a in concat_in]
    return f, dev_args


def _build_tiny(n_cores):
    nc = bacc.Bacc("TRN2", target_bir_lowering=False, debug=False,
                   num_devices=n_cores)
    tok = nc.declare_dram_parameter("tok", [TILE, 8], F32, isOutput=False)
    tok_out = nc.declare_dram_parameter("tok_out", [TILE, 8], F32,
                                        isOutput=True)
    with tile.TileContext(nc) as tc, ExitStack() as ctx:
        pool = ctx.enter_context(tc.tile_pool(name="p", bufs=1))
        t = pool.tile([TILE, 8], F32)
        nc.sync.dma_start(t[:], tok[:])
        nc.sync.dma_start(tok_out[:], t[:])
    nc.compile()
    return nc


def _timed_slope(f, dev_args, reps=9, k_lo=2, k_hi=10):
    """Per-exec device time via slope fitting: wall time of k_hi chained
    executions minus wall time of k_lo, divided by (k_hi - k_lo). Chained
    async dispatches execute back-to-back on-device, so the tunnel
    round-trip and dispatch overhead cancel in the difference."""
    import jax
    import time as _t

    def run_k(k):
        outs = None
        t0 = _t.perf_counter()
        for _ in range(k):
            outs = f(*dev_args)
        jax.block_until_ready(outs)
        return _t.perf_counter() - t0

    run_k(2)
    run_k(2)
    lo, hi = [], []
    for _ in range(reps):
        lo.append(run_k(k_lo))
        hi.append(run_k(k_hi))
    lo.sort(); hi.sort()
    med_lo = lo[len(lo) // 2]
    med_hi = hi[len(hi) // 2]
    return (med_hi - med_lo) / (k_hi - k_lo), med_lo, med_hi


def measure_hw_time(inputs, reps=30, cfg=None, stage=99):
    """Per-execution device time estimate: wall time of the kernel with
    device-resident inputs, minus the same measurement for a trivial
    pass-through program (dispatch/tunnel baseline)."""
    cfg = cfg or Cfg()
    cs, t_pad, per_core, g1rows = host_prep(inputs, cfg)
    key = (cfg.N, t_pad, tuple(cs), tuple(g1rows), stage)
    if key not in _CACHE:
        _CACHE[key] = build_program(cfg, cs, t_pad, g1rows, stage=stage)
    nc = _CACHE[key]
    in_maps = make_in_maps(inputs, cfg, per_core, t_pad)
    f, dev_args = _exec_maker(nc, in_maps, cfg.NC)
    per_exec, med_lo, med_hi = _timed_slope(f, dev_args)
    print(f"slope fit: k2={med_lo*1e3:.2f}ms k10={med_hi*1e3:.2f}ms "
          f"-> per-exec {per_exec*1e3:.3f} ms")
    return per_exec * 1e9


# revision 56
# speedup vs baseline: 1.4232x; 1.4232x over previous
"""Trainium2 Bass kernel for a 2-layer GAT + attention pooling (PyG-style).

Strategy (8 NeuronCores, SPMD):
  * Destination nodes are range-partitioned: core k owns dst nodes
    [k*N/8, (k+1)*N/8). Edges (self-loops appended) are sorted by dst and
    routed to the owning core (host-side index prep only).
  * conv1's dense part (h1 = x @ W1.T, per-node a_src) is computed
    replicated on every core (bf16) into a packed DRAM table:
    row(n) = [h1(n) | a_src(n)], row stride 640 bf16 (dma_gather needs
    256B-multiple rows).
  * Per-edge work: one dma_gather of table rows by src id, spread over
    4 SWDGE queues (single-queue gathers serialize call-by-call on real
    hw, ~25x slower for small rows). Within each chunk, edges are sorted
    by (src half, dst) and each gather call's table operand is sliced to
    the row prefix it needs, so conv1 gathers overlap the ext1 build.
  * The dst-side attention values are expanded per edge with two tiny
    matmuls through the <=32 distinct dsts per 128-edge tile (host
    ships compact selectors; per-tile mm1s write a block-diagonal small
    matrix, one whole-window mm2 expands 3 tiles at once), avoiding PE
    transposes and big PSUM->SBUF selector copies.
  * Weighted segment-sum rides the PE via the per-tile one-hot selector
    (built on-chip from shipped dloc values), accumulating numerator and
    denominator in PSUM per dst chunk of 128 nodes. Edge tiles are
    allocated per chunk (variable count) to minimise padding.
  * conv2 table rows are 128-col (h2 only; attention dots computed
    on-chip via a basis fold); the inter-conv AllGather is split into 3
    node-range collectives issued as chunks finish so they overlap the
    conv1 edge phase (per-collective overhead ~30us on hw dominates
    payload). The final head is folded per-core before a tiny [256,2]
    AllGather + on-chip sum (cheaper than AllReduce) of graph-pooling
    partials.
"""

import numpy as np
from contextlib import ExitStack

import concourse.bass as bass
import concourse.tile as tile
from concourse import bacc, mybir

F32 = mybir.dt.float32
F32R = mybir.dt.float32r
BF16 = mybir.dt.bfloat16
U8 = mybir.dt.uint8
I16 = mybir.dt.int16
AL = mybir.AluOpType
AF = mybir.ActivationFunctionType

TILE = 128
NEG = 0.2
KSL = 32  # selector slots per tile (max distinct dsts per 128-edge tile)
NSPLIT = 3  # AllGather splits (by chunk ranges)
SP_CHUNKS = [0, 9, 16, 20]  # split boundaries in chunks


class Cfg:
    def __init__(self, N=20000, NC=8, GRP=32, H1=8, C1=64, FIN=128, D2=128,
                 NGR=256, OUT=2):
        self.N, self.NC, self.GRP = N, NC, GRP
        self.H1, self.C1, self.FIN, self.D2 = H1, C1, FIN, D2
        self.D1 = H1 * C1
        self.NGR, self.OUT = NGR, OUT
        self.NLOC = N // NC
        self.CH = (self.NLOC + TILE - 1) // TILE
        self.EXT1 = 640                  # conv1 table row stride (bf16)
        self.ROW1 = self.D1 + H1        # written cols: h | a_src
        self.NT1 = (N + TILE - 1) // TILE
        chs = SP_CHUNKS
        assert chs[-1] == self.CH and len(chs) == NSPLIT + 1
        self.sp_ch = chs
        self.sp_rows = [min(self.NLOC, chs[s + 1] * TILE) -
                        min(self.NLOC, chs[s] * TILE) for s in range(NSPLIT)]
        self.sp_r0 = [min(self.NLOC, chs[s] * TILE) for s in range(NSPLIT)]
        self.sp_base = np.cumsum([0] + [r * NC for r in self.sp_rows]).tolist()


def build_program(cfg, cs, t_pad, g1rows=None, stage=99, dbg=False):
    """cs: per-chunk tile start offsets (len CH+1); cs[-1] extended to t_pad
    by folding trailing pad tiles into the last chunk. g1rows: per
    gather-call ext1 row prefix for sliced table dependencies."""
    c_ = cfg
    ng = t_pad // c_.GRP
    nt1 = c_.NT1

    nc = bacc.Bacc("TRN2", target_bir_lowering=False, debug=False,
                   num_devices=c_.NC, num_swdge_queues=4)

    def par(name, shape, dt=F32):
        return nc.declare_dram_parameter(name, shape, dt, isOutput=False)

    smw = c_.D1 + 4 * c_.D2 + TILE + c_.OUT + 4   # smalls width
    xT_p = par("x", [c_.FIN, nt1 * TILE], BF16)    # x transposed bf16 (host)
    xlocT_p = par("xloc", [c_.FIN, c_.CH * TILE], BF16)
    w1tb = par("w1tb", [c_.FIN, c_.D1], BF16)      # W1.T bf16
    wpa = par("wpa", [c_.D2, c_.D1])               # w2 (f32)
    wpb = par("wpb", [2 * c_.D1, c_.D2])           # w1 | w2t@B2
    apk = par("apk", [c_.D1, 2 * c_.H1 + 4])       # a1m | a2m | woutt
    gidx = par("gidx", [TILE, 2 * t_pad * 8], I16)  # gsrc1 | gsrc2
    dlp = par("dlp", [TILE, t_pad + TILE], BF16)   # dloc | iotaF
    selp = par("selp", [TILE, c_.CH * c_.NGR], BF16)  # b01 pooling sel
    nwin = -(-c_.GRP // 3)  # 3-tile selector windows per group
    selb = par("selb", [TILE, ng * nwin * TILE], BF16)  # b32 per window
    selt = par("selt", [TILE, ng * nwin * TILE], BF16)  # sel32T windows
    smalls = par("smalls", [TILE, smw])
    tok = par("tok", [TILE, 8])
    w2 = wpa
    w1 = wpb[0:c_.D1, :]
    w2t = wpb[c_.D1:2 * c_.D1, :]
    a1m = apk[:, 0:2 * c_.H1]
    a2m = apk[0:c_.D2, 2 * c_.H1:2 * c_.H1 + 2]
    woutt = apk[0:c_.D2, 2 * c_.H1 + 2:2 * c_.H1 + 4]
    gsrc1 = gidx[:, 0:t_pad * 8]
    gsrc2 = gidx[:, t_pad * 8:2 * t_pad * 8]
    o_ = [0]
    def sl(w):
        a = o_[0]; o_[0] += w
        return smalls[:, a:a + w]
    b1r = sl(c_.D1)
    b2r = sl(c_.D2)
    wattnr = sl(c_.D2)
    wmaskr = sl(c_.D2)
    as2r = sl(c_.D2)
    inv2 = sl(1)
    ident = sl(TILE)
    boutr = sl(c_.OUT)
    battn = sl(1)
    epsr = sl(1)
    bmask = sl(1)

    out_p = nc.declare_dram_parameter("out", [c_.NGR, c_.OUT], F32,
                                      isOutput=True)
    dbgh_p = nc.declare_dram_parameter(
        "dbgh", [TILE, c_.CH * c_.D1], BF16, isOutput=True) if dbg else None
    dbgo_p = nc.declare_dram_parameter(
        "dbgo", [TILE, c_.CH * c_.D2], F32, isOutput=True) if dbg else None
    tok_out = nc.declare_dram_parameter("tok_out", [TILE, 8], F32,
                                        isOutput=True)

    ext1 = nc.dram_tensor("ext1", [nt1 * TILE, c_.EXT1], BF16)
    ag_in = nc.dram_tensor("ag_in", [c_.NLOC, c_.D2], BF16)
    ext2 = nc.dram_tensor("ext2", [c_.N, c_.D2], BF16, addr_space="Shared")
    ar_in = nc.dram_tensor("ar_in", [c_.NGR, c_.OUT], F32)
    agred = nc.dram_tensor("agred", [c_.NC * c_.NGR, c_.OUT], F32,
                           addr_space="Shared")

    chunk_start = [int(cs[c]) for c in range(c_.CH)]
    chunk_end = [int(cs[c + 1]) for c in range(c_.CH)]
    chunk_end[c_.CH - 1] = t_pad
    chunk_of = np.zeros(t_pad, np.int32)
    for c in range(c_.CH):
        chunk_of[chunk_start[c]:chunk_end[c]] = c

    # a_dst-expand windows: 3 tiles share a 128-partition selector block
    # (32 slots each at bases 0/32/64). Per-tile mm1s write a
    # block-diagonal [slots, 3*nh] small matrix (each tile's rhs variant
    # zero-pads the other bands), so a single whole-window mm2 expands
    # all 3 tiles at once. Windows never cross group boundaries.
    win_by_group = []
    for g in range(ng):
        wins = []
        for wl in range(nwin):
            t0 = g * c_.GRP + wl * 3
            nt = min(3, (g + 1) * c_.GRP - t0)
            rr = [(i, int(chunk_of[t0 + i])) for i in range(nt)]
            wins.append((wl, t0, nt, rr))
        win_by_group.append(wins)

    d1c = c_.D1 // TILE   # f-chunks of D1
    g_half = c_.NGR // 2
    SB = 8                # sub-group tiles (dependency granularity)
    XB0 = 8               # x tiles per DMA

    with tile.TileContext(nc) as tc, ExitStack() as ctx:
        const = ctx.enter_context(tc.tile_pool(name="const", bufs=1))
        io = ctx.enter_context(tc.tile_pool(name="io", bufs=3))
        gat = ctx.enter_context(tc.tile_pool(name="gat", bufs=2))
        sm = ctx.enter_context(tc.tile_pool(name="sm", bufs=2))
        sel = ctx.enter_context(tc.tile_pool(name="sel", bufs=2))
        pp = ctx.enter_context(tc.tile_pool(name="pp", bufs=2, space="PSUM"))
        ppt = ctx.enter_context(tc.tile_pool(name="ppt", bufs=2, space="PSUM"))
        pc2 = ctx.enter_context(tc.tile_pool(name="pc2", bufs=1, space="PSUM"))

        _lc = [0]
        def load_const(p, shape, dt=F32):
            _lc[0] += 1
            t = const.tile(shape, dt, tag=f"c_{_lc[0]}")
            nc.sync.dma_start(t[:], p if isinstance(p, bass.AP) else p[:])
            return t

        ident_t = load_const(ident, [TILE, TILE])
        tok_t = const.tile([TILE, 8], F32, tag="tok")
        nc.sync.dma_start(tok_t[:], tok[:])
        nc.sync.dma_start(tok_out[:], tok_t[:])
        w1t_t = load_const(w1tb, [c_.FIN, c_.D1], BF16)
        woutt_t = load_const(woutt, [c_.D2, c_.OUT])
        b2r_t = load_const(b2r, [TILE, c_.D2])
        boutr_t = load_const(boutr, [TILE, c_.OUT])
        wattn_t = load_const(wattnr, [TILE, c_.D2])
        wmask_t = load_const(wmaskr, [TILE, c_.D2])
        battn_t = load_const(battn, [TILE, 1])
        eps_t = load_const(epsr, [TILE, 1])
        bmask_t = load_const(bmask, [TILE, 1])
        gsrc1_t = load_const(gsrc1, [TILE, t_pad * 8], I16)
        gsrc2_t = load_const(gsrc2, [TILE, t_pad * 8], I16)
        a2m_t = load_const(a2m, [c_.D2, 2])
        w2_t = load_const(w2, [c_.D2, c_.D1])
        dloc_t = load_const(dlp[:, 0:t_pad], [TILE, t_pad], BF16)
        iotaF_t = load_const(dlp[:, t_pad:t_pad + TILE], [TILE, TILE], BF16)
        b1r_f = load_const(b1r, [TILE, c_.D1])
        as2mod_t = load_const(as2r, [TILE, c_.D2])
        inv2_t = load_const(inv2, [TILE, 1])

        w2t_t = const.tile([TILE, d1c, c_.D2], F32)
        a1m_t = const.tile([TILE, d1c, 2 * c_.H1], F32)
        w1_t = const.tile([TILE, d1c, c_.FIN], F32)
        for dc in range(d1c):
            nc.sync.dma_start(w2t_t[:, dc, :], w2t[dc * TILE:(dc + 1) * TILE, :])
            nc.sync.dma_start(a1m_t[:, dc, :], a1m[dc * TILE:(dc + 1) * TILE, :])
            nc.sync.dma_start(w1_t[:, dc, :], w1[dc * TILE:(dc + 1) * TILE, :])

        # bf16 copies of conv2 weights / conv1 bias
        w2t_bf = const.tile([TILE, d1c, c_.D2], BF16)
        for dc in range(d1c):
            nc.vector.tensor_copy(w2t_bf[:, dc, :], w2t_t[:, dc, :])
        b1r_t = const.tile([TILE, c_.D1], BF16)
        nc.vector.tensor_copy(b1r_t[:], b1r_f[:])
        ident_bf = const.tile([TILE, TILE], BF16)
        nc.vector.tensor_copy(ident_bf[:], ident_t[:])

        # A1eff = W1.T @ A1m  [FIN, 2*H1];  A2eff = W2.T @ A2m  [D1, 2]
        a1eff_ps = ppt.tile([c_.FIN, 2 * c_.H1], F32, tag="tps")
        for dc in range(d1c):
            nc.tensor.matmul(a1eff_ps[:], w1_t[:, dc, :], a1m_t[:, dc, :],
                             start=(dc == 0), stop=(dc == d1c - 1))
        a1eff = const.tile([c_.FIN, 2 * c_.H1], BF16)
        nc.vector.tensor_copy(a1eff[:], a1eff_ps[:])

        a2eff = const.tile([TILE, d1c, 2], BF16)
        for fc in range(d1c):
            a2eff_ps = ppt.tile([TILE, 2], F32, tag="tps")
            nc.tensor.matmul(a2eff_ps[:], w2_t[:, fc * TILE:(fc + 1) * TILE],
                             a2m_t[:], start=True, stop=True)
            nc.vector.tensor_copy(a2eff[:, fc, :], a2eff_ps[:])

        # local a_dst tables, 3 banded variants each (band a holds the
        # values, other bands zero) for block-diagonal window mm1s:
        # conv1 [TILE, CH, 3*H1] x3, conv2 [TILE, CH, 3] x3
        adloc1 = [const.tile([TILE, c_.CH, 3 * c_.H1], BF16,
                             name=f"adloc1_{a}", tag=f"adloc1_{a}")
                  for a in range(3)]
        adloc2 = [const.tile([TILE, c_.CH, 3], BF16, name=f"adloc2_{a}",
                             tag=f"adloc2_{a}") for a in range(3)]
        for a in range(3):
            nc.vector.memset(adloc1[a][:], 0.0)
            nc.vector.memset(adloc2[a][:], 0.0)
        for cb in range((c_.CH + XB0 - 1) // XB0):
            nb = min(XB0, c_.CH - cb * XB0)
            xl = io.tile([c_.FIN, XB0 * TILE], BF16, tag="xt")
            nc.sync.dma_start(xl[:, 0:nb * TILE],
                              xlocT_p[:, cb * XB0 * TILE:
                                      (cb * XB0 + nb) * TILE])
            for i in range(nb):
                c = cb * XB0 + i
                ad_ps = ppt.tile([TILE, c_.H1], F32, tag="tps")
                nc.tensor.matmul(ad_ps[:], xl[:, i * TILE:(i + 1) * TILE],
                                 a1eff[:, c_.H1:2 * c_.H1],
                                 start=True, stop=True)
                for a in range(3):
                    nc.vector.tensor_copy(
                        adloc1[a][:, c, a * c_.H1:(a + 1) * c_.H1], ad_ps[:])

        # =========== conv1 phase 1: replicated ext1 table ===========
        xt4 = None
        stg = None
        for t in range(nt1 if stage >= 1 else 0):
            if t % XB0 == 0:
                nb = min(XB0, nt1 - t)
                xt4 = io.tile([c_.FIN, XB0 * TILE], BF16, tag="xt")
                nc.sync.dma_start(xt4[:, 0:nb * TILE],
                                  xT_p[:, t * TILE:(t + nb) * TILE])
            xt = xt4[:, (t % XB0) * TILE:(t % XB0 + 1) * TILE]
            h_ps = pp.tile([TILE, c_.ROW1], F32, tag="pnum")
            nc.tensor.matmul(h_ps[:, 0:c_.D1], xt, w1t_t[:],
                             start=True, stop=True)
            nc.tensor.matmul(h_ps[:, c_.D1:c_.ROW1], xt, a1eff[:, 0:c_.H1],
                             start=True, stop=True)
            if t % 4 == 0:
                stg = io.tile([TILE, 4, c_.ROW1], BF16, tag="stg")
            sv = stg[:, t % 4, :]
            if t % 2 == 0:
                nc.scalar.copy(sv[:], h_ps[:])
            else:
                nc.vector.tensor_copy(sv[:], h_ps[:])
            if t % 4 == 3 or t == nt1 - 1:
                t0w = t - (t % 4)
                nj = t - t0w + 1
                dst = ext1[t0w * TILE:(t0w + nj) * TILE, 0:c_.ROW1].rearrange(
                    "(j p) e -> p j e", p=TILE)
                nc.sync.dma_start(dst, stg[:, 0:nj, :])

        h_own = const.tile([TILE, c_.CH, c_.D1], BF16)
        o2_own = const.tile([TILE, c_.CH, c_.D2], F32)

        # =========== conv2 phase 1 for a chunk + AG split ===========
        def conv2_p1(c):
            h2_ps = pc2.tile([TILE, c_.D2], F32, tag="c2h")
            a2_ps = pc2.tile([TILE, 2], F32, tag="c2a")
            for fc in range(d1c):
                hT_ps = pc2.tile([TILE, TILE], BF16, tag="hTp")
                nc.tensor.transpose(
                    hT_ps[:], h_own[:, c, fc * TILE:(fc + 1) * TILE],
                    ident_bf[:])
                hT = io.tile([TILE, TILE], BF16, tag="hT")
                nc.vector.tensor_copy(hT[:], hT_ps[:])
                nc.tensor.matmul(h2_ps[:], hT[:], w2t_bf[:, fc, :],
                                 start=(fc == 0), stop=(fc == d1c - 1))
                nc.tensor.matmul(a2_ps[:], hT[:],
                                 a2eff[:, fc, :],
                                 start=(fc == 0), stop=(fc == d1c - 1))
            stg2 = io.tile([TILE, c_.D2], BF16, tag="stg2")
            nc.vector.tensor_copy(stg2[:], h2_ps[:])
            for a in range(3):
                nc.vector.tensor_copy(adloc2[a][:, c, a:a + 1],
                                      a2_ps[:, 1:2])
            rows = min(TILE, c_.NLOC - c * TILE)
            nc.sync.dma_start(ag_in[c * TILE:c * TILE + rows, :],
                              stg2[0:rows, :])

        def ag_split(s):
            r0, rws = c_.sp_r0[s], c_.sp_rows[s]
            nc.gpsimd.collective_compute(
                "AllGather", AL.bypass, replica_groups=[list(range(c_.NC))],
                ins=[ag_in[r0:r0 + rws, :]],
                outs=[ext2[c_.sp_base[s]:c_.sp_base[s + 1], :]])

        # =========== shared edge-aggregation pipeline ===========
        def conv_phase2(conv, do_split):
            # conv==1: ext1 rows (640 stride), nh=H1, dfeat=D1
            # conv==2: ext2 rows (128 stride), nh=1, dfeat=D2
            if conv == 1:
                nh, dfeat, adloc, bias_t, dst_sb = \
                    c_.H1, c_.D1, adloc1, b1r_t, h_own
                gs_t = gsrc1_t
            else:
                nh, dfeat, adloc, bias_t, dst_sb = \
                    1, c_.D2, adloc2, b2r_t, None
                gs_t = gsrc2_t
            psn = None
            next_split = [0]
            cph = dfeat // nh

            GH = c_.GRP // 4  # quarter-group gather granularity
            def issue_gather(g):
                if conv == 1:
                    tab, cols = ext1, c_.EXT1
                    extg = gat.tile([TILE, c_.GRP, c_.EXT1], BF16, tag="extg")
                else:
                    tab, cols = ext2, c_.D2
                    extg = gat.tile([TILE, c_.GRP, c_.D2], BF16, tag="extg")
                for hh in range(4):
                    if conv == 1 and g1rows is not None:
                        rows = int(g1rows[g * 4 + hh])
                    else:
                        rows = tab.shape[0]
                    nc.gpsimd.dma_gather(
                        extg[:, hh * GH:(hh + 1) * GH, :], tab[0:rows, 0:cols],
                        gs_t[:, (g * c_.GRP + hh * GH) * 8:
                             (g * c_.GRP + (hh + 1) * GH) * 8],
                        GH * TILE, GH * TILE, cols,
                        elem_step=cols, single_packet=(conv == 1),
                        queue_num=hh)
                return extg

            nxt = issue_gather(0)
            for g in range(ng):
                extg = nxt
                if g + 1 < ng:
                    nxt = issue_gather(g + 1)

                adt = BF16 if conv == 1 else F32
                s01g = sel.tile([TILE, c_.GRP, TILE], BF16, tag="s01g")
                b8g = sel.tile([TILE, nwin, TILE], BF16, tag="b8g")
                sel8g = sel.tile([TILE, nwin, TILE], BF16, tag="sel8g")
                nc.sync.dma_start(
                    b8g[:], selb[:, g * nwin * TILE:(g + 1) * nwin * TILE]
                    .rearrange("p (b e) -> p b e", e=TILE))
                nc.sync.dma_start(
                    sel8g[:], selt[:, g * nwin * TILE:(g + 1) * nwin * TILE]
                    .rearrange("p (b e) -> p b e", e=TILE))
                advg = sm.tile([TILE, c_.GRP, nh], adt, tag="advg")
                pg = sm.tile([TILE, c_.GRP, nh], BF16, tag="pg")
                if conv == 2:
                    sa_t = sm.tile([TILE, c_.GRP, 1], F32, tag="sa2")
                dld = sm.tile([TILE, c_.GRP, 2], BF16, tag="dld")
                nc.scalar.copy(dld[:], dloc_t[:, g * c_.GRP:(g + 1) * c_.GRP]
                               .unsqueeze(2).broadcast_to([TILE, c_.GRP, 2]))
                # --- selector build via iota compare (batched)
                EQB = 4
                for b in range(c_.GRP // EQB):
                    bo = b * EQB
                    dl = dld[:, bo:bo + EQB, :].unsqueeze(2).broadcast_to(
                        [TILE, EQB, TILE // 2, 2])
                    io_f = iotaF_t[:].rearrange("p (a b) -> p a b", b=2) \
                        .unsqueeze(1).broadcast_to([TILE, EQB, TILE // 2, 2])
                    s01v = s01g[:, bo:bo + EQB, :].rearrange(
                        "p g (a b) -> p g a b", b=2)
                    nc.vector.tensor_tensor(s01v, io_f, dl, AL.is_equal)

                # --- a_dst expand: per-tile mm1s fill a block-diagonal
                # [slots, 3*nh] small matrix; one whole-window mm2 expands
                # all 3 tiles' a_dst values at once.
                for (wl, t0, nt, rr) in win_by_group[g]:
                    tl0 = t0 % c_.GRP
                    small_ps = ppt.tile([TILE, 3 * nh], F32, tag="tps")
                    for (a, c) in rr:
                        nc.tensor.matmul(
                            small_ps[a * KSL:(a + 1) * KSL, 0:nt * nh],
                            b8g[:, wl, a * KSL:(a + 1) * KSL],
                            adloc[a][:, c, 0:nt * nh],
                            start=True, stop=True)
                    small_sb = sm.tile([TILE, 3 * nh], BF16, tag="smsb")
                    nc.scalar.copy(small_sb[0:nt * KSL, 0:nt * nh],
                                   small_ps[0:nt * KSL, 0:nt * nh])
                    adv_ps = ppt.tile([TILE, 3, nh], F32, tag="tps")
                    nc.tensor.matmul(
                        adv_ps[:, 0:nt, :].rearrange("p a h -> p (a h)"),
                        sel8g[0:nt * KSL, wl, :],
                        small_sb[0:nt * KSL, 0:nt * nh],
                        start=True, stop=True)
                    nc.scalar.copy(advg[:, tl0:tl0 + nt, :],
                                   adv_ps[:, 0:nt, :])

                for sub in range(c_.GRP // SB):
                    sl_ = slice(sub * SB, (sub + 1) * SB)
                    if conv == 1:
                        sa = extg[:, sl_, c_.D1:c_.D1 + nh]
                        nc.vector.tensor_tensor(sa, sa, advg[:, sl_, :],
                                                AL.add)
                    else:
                        # a_src2 rides gathered col 0 (basis-folded on host)
                        sa = sa_t[:, sl_, :]
                        nc.vector.tensor_tensor(sa, extg[:, sl_, 0:1],
                                                advg[:, sl_, :], AL.add)
                    tmp = sm.tile([TILE, SB, nh], adt, tag="tmp")
                    nc.scalar.mul(tmp[:], sa, NEG)
                    nc.vector.tensor_tensor(sa, sa, tmp[:], AL.max)
                    nc.scalar.activation(pg[:, sl_, :], sa, AF.Exp)
                    # duplicate alpha x2 so the multiply's operands are all
                    # packed in the last dim (DVE 2x mode)
                    pexp = sm.tile([TILE, SB, nh, 2], BF16, tag="pexp")
                    nc.scalar.copy(pexp[:], pg[:, sl_, :].unsqueeze(
                        3).broadcast_to([TILE, SB, nh, 2]))

                    # --- alpha-weight the gathered rows (packed-pair view)
                    ev = extg[:, sl_, 0:dfeat].rearrange(
                        "p g (h a b) -> p g h a b", h=nh, b=2)
                    pb = pexp[:].unsqueeze(3).broadcast_to(
                        [TILE, SB, nh, cph // 2, 2])
                    nc.vector.tensor_tensor(ev, ev, pb, AL.mult)

                    # --- segment-sum via selector matmuls
                    for i in range(SB):
                        tl = sub * SB + i
                        t = g * c_.GRP + tl
                        c = int(chunk_of[t])
                        first = t == chunk_start[c]
                        last = t == chunk_end[c] - 1
                        if first:
                            psn = pp.tile([TILE, 512 + nh], F32, tag="pnum")
                        lhs = s01g[:, tl, :]
                        nc.tensor.matmul(psn[:, 0:dfeat], lhs,
                                         extg[:, tl, 0:dfeat],
                                         start=first, stop=last)
                        nc.tensor.matmul(psn[:, 512:512 + nh], lhs,
                                         pg[:, tl, :],
                                         start=first, stop=last)
                        if not last:
                            continue
                        den = sm.tile([TILE, nh], F32, tag="den")
                        nc.vector.tensor_scalar_add(den[:],
                                                    psn[:, 512:512 + nh],
                                                    1e-16)
                        denr = sm.tile([TILE, nh], F32, tag="denr")
                        nc.vector.reciprocal(denr[:], den[:])
                        if conv == 1:
                            for h in range(nh):
                                nc.scalar.activation(
                                    dst_sb[:, c, h * cph:(h + 1) * cph],
                                    psn[:, h * cph:(h + 1) * cph], AF.Copy,
                                    scale=denr[:, h:h + 1])
                            nc.vector.tensor_tensor(
                                dst_sb[:, c, :], dst_sb[:, c, :], bias_t[:],
                                AL.add)
                            nc.vector.tensor_scalar_max(
                                dst_sb[:, c, :], dst_sb[:, c, :], 0.0)
                            if do_split and next_split[0] < NSPLIT and \
                                    c + 1 == c_.sp_ch[next_split[0] + 1]:
                                s = next_split[0]
                                for cc in range(c_.sp_ch[s], c_.sp_ch[s + 1]):
                                    conv2_p1(cc)
                                ag_split(s)
                                next_split[0] += 1
                        else:
                            o2 = o2_own[:, c, :]
                            nc.scalar.activation(o2, psn[:, 0:dfeat], AF.Copy,
                                                 scale=denr[:])
                            # undo the a_src2 basis fold on column 0:
                            # h2[0] = (M0 - sum_j>0 as2[j]*M[j]) / as2[0]
                            scr2 = sm.tile([TILE, c_.D2], F32, tag="scr2")
                            acc2 = sm.tile([TILE, 1], F32, tag="acc2")
                            nc.vector.tensor_tensor(scr2[:], o2, as2mod_t[:],
                                                    AL.mult)
                            nc.vector.tensor_reduce(
                                acc2[:], scr2[:], mybir.AxisListType.X, AL.add)
                            t1 = sm.tile([TILE, 1], F32, tag="t1c")
                            nc.vector.tensor_tensor(t1[:], o2_own[:, c, 0:1],
                                                    acc2[:], AL.subtract)
                            nc.scalar.activation(o2_own[:, c, 0:1], t1[:],
                                                 AF.Copy, scale=inv2_t[:])
                            nc.vector.tensor_tensor(o2, o2, bias_t[:], AL.add)
                            nc.vector.tensor_scalar_max(o2, o2, 0.0)

        if stage >= 2:
            conv_phase2(1, do_split=(stage >= 3))
        else:
            nc.vector.memset(h_own[:], 0.0)

        if stage == 2:
            for cc in range(c_.CH):
                conv2_p1(cc)

        # =========== conv2 phase 2 ===========
        if stage >= 4:
            conv_phase2(2, do_split=False)
        else:
            nc.vector.memset(o2_own[:], 0.0)

        if dbg:
            nc.sync.dma_start(dbgh_p[:, :],
                              h_own[:].rearrange("p c d -> p (c d)"))
            nc.sync.dma_start(dbgo_p[:, :],
                              o2_own[:].rearrange("p c d -> p (c d)"))

        # =========== pooling partials + folded head + tiny AllReduce =====
        ps_pa = pp.tile([TILE, c_.D2], F32, tag="pnum")
        ps_pb = pc2.tile([TILE, c_.D2], F32, tag="c2h")
        nch_pool = c_.CH if stage >= 5 else 1
        for c in range(nch_pool):
            h = o2_own[:, c, :]
            ta = sm.tile([TILE, c_.D2], F32, tag="ta")
            nc.vector.tensor_tensor(ta[:], h, wattn_t[:], AL.mult)
            sa = sm.tile([TILE, 1], F32, tag="sa")
            nc.vector.tensor_reduce(sa[:], ta[:], mybir.AxisListType.X, AL.add)
            nc.scalar.activation(sa[:], sa[:], AF.Identity, bias=battn_t[:])
            tm = sm.tile([TILE, c_.D2], F32, tag="ta")
            nc.vector.tensor_tensor(tm[:], h, wmask_t[:], AL.mult)
            sb = sm.tile([TILE, 1], F32, tag="sb")
            nc.vector.tensor_reduce(sb[:], tm[:], mybir.AxisListType.X, AL.add)
            nc.scalar.activation(sb[:], sb[:], AF.Sigmoid, bias=bmask_t[:])
            fac = sm.tile([TILE, 1], F32, tag="fac")
            nc.vector.tensor_tensor(fac[:], sa[:], sb[:], AL.mult)
            wn = sm.tile([TILE, c_.D2], BF16, tag="wn")
            nc.scalar.activation(wn[:], h, AF.Copy, scale=fac[:])
            b01g = sm.tile([TILE, c_.NGR], BF16, tag="b01g")
            nc.sync.dma_start(b01g[:], selp[:, c * c_.NGR:(c + 1) * c_.NGR])
            nc.tensor.matmul(ps_pa[:], b01g[:, 0:g_half], wn[:],
                             start=(c == 0), stop=(c == nch_pool - 1))
            nc.tensor.matmul(ps_pb[:], b01g[:, g_half:c_.NGR], wn[:],
                             start=(c == 0), stop=(c == nch_pool - 1))
        # per-core folded head: partial_out = pooled_partial @ W_out.T
        for half, ps in ((0, ps_pa), (1, ps_pb)):
            pl = io.tile([g_half, c_.D2], F32, tag="pl")
            nc.vector.tensor_copy(pl[:], ps[0:g_half, :])
            pT_ps = ppt.tile([c_.D2, g_half], F32, tag="tps")
            nc.tensor.transpose(pT_ps[:], pl[:], ident_t[0:g_half, 0:g_half])
            pT = io.tile([c_.D2, g_half], F32, tag="xTp")
            nc.vector.tensor_copy(pT[:], pT_ps[:])
            o_ps = ppt.tile([g_half, c_.OUT], F32, tag="tps")
            nc.tensor.matmul(o_ps[:], pT[:], woutt_t[:], start=True, stop=True)
            ot = io.tile([g_half, c_.OUT], F32, tag="ot")
            nc.vector.tensor_copy(ot[:], o_ps[:])
            nc.sync.dma_start(ar_in[half * g_half:(half + 1) * g_half, :],
                              ot[:])

        # AllGather the per-core [256,2] head partials, sum on-chip
        # (cheaper than an AllReduce on real hw), add b_out once.
        of_all = io.tile([g_half, 2 * c_.NC, c_.OUT], F32, tag="of")
        if stage >= 1:
            nc.gpsimd.collective_compute(
                "AllGather", AL.bypass, replica_groups=[list(range(c_.NC))],
                ins=[ar_in[:]], outs=[agred[:]])
            nc.sync.dma_start(
                of_all[:], agred[:].rearrange("(b p) e -> p b e", p=g_half))
        else:
            nc.vector.memset(of_all[:], 0.0)
            nc.sync.dma_start(
                of_all[:, 0:2, :],
                ar_in[:].rearrange("(b p) e -> p b e", p=g_half))
        acc = io.tile([g_half, 2, c_.OUT], F32, tag="ofacc")
        nc.vector.tensor_tensor(acc[:], of_all[:, 0:2, :],
                                of_all[:, 2:4, :], AL.add)
        for k in range(2, c_.NC):
            nc.vector.tensor_tensor(acc[:], acc[:],
                                    of_all[:, 2 * k:2 * k + 2, :], AL.add)
        nc.vector.tensor_tensor(
            acc[:], acc[:],
            boutr_t[0:g_half, :].unsqueeze(1).broadcast_to(
                [g_half, 2, c_.OUT]), AL.add)
        nc.sync.dma_start(out_p[:].rearrange("(j p) e -> p j e", p=g_half),
                          acc[:])

    nc.compile()
    return nc


def host_prep(inputs, cfg):
    c_ = cfg
    ei = np.asarray(inputs["edge_index"], np.int64)
    batch = np.asarray(inputs["batch"], np.int64)

    loops = np.arange(c_.N, dtype=np.int64)
    src = np.concatenate([ei[0], loops])
    dst = np.concatenate([ei[1], loops])
    order = np.argsort(dst, kind="stable")
    src_s = src[order]
    dst_s = dst[order]

    # within each (core, chunk): order edges by (src half, dst) so early
    # tiles only reference the first half of the ext1 table -> conv1
    # gathers can start while ext1 is still being written
    lo_hi = []
    tiles_c = np.zeros(c_.CH, np.int64)
    for k in range(c_.NC):
        row = []
        for c in range(c_.CH):
            d0 = k * c_.NLOC + c * TILE
            d1 = min(k * c_.NLOC + c_.NLOC, d0 + TILE)
            lo = int(np.searchsorted(dst_s, d0))
            hi = int(np.searchsorted(dst_s, d1))
            row.append((lo, hi, d0))
            tiles_c[c] = max(tiles_c[c], -(-(hi - lo) // TILE))
        lo_hi.append(row)
    cs = np.concatenate([[0], np.cumsum(tiles_c)])
    t_pad = -(-int(cs[-1]) // c_.GRP) * c_.GRP

    # ext2 (split-permuted) row index for a global node id
    sp_r0 = np.asarray(c_.sp_r0 + [c_.NLOC], np.int64)
    sp_base = np.asarray(c_.sp_base, np.int64)
    sp_rows = np.asarray(c_.sp_rows, np.int64)

    def ext2_row(n):
        core = n // c_.NLOC
        pos = n % c_.NLOC
        s = np.searchsorted(sp_r0, pos, side="right") - 1
        return sp_base[s] + core * sp_rows[s] + (pos - sp_r0[s])

    def wrap_idx(a):
        w = a.reshape(-1, 16).T.astype(np.int16)
        return np.tile(w, (8, 1)).copy()

    import ml_dtypes
    per_core = []
    for k in range(c_.NC):
        gi_src = np.zeros(t_pad * TILE, np.int64)
        dloc = np.full(t_pad * TILE, -1, np.int64)
        for c in range(c_.CH):
            lo, hi, d0 = lo_hi[k][c]
            cnt = hi - lo
            j = int(cs[c]) * TILE + np.arange(cnt)
            ss = src_s[lo:hi]
            dd = dst_s[lo:hi] - d0
            o2 = np.lexsort((dd, ss * 2 // c_.N))
            gi_src[j] = ss[o2]
            dloc[j] = dd[o2]
        gi_src2 = ext2_row(gi_src)
        # dloc in [TILE, t_pad] layout: col t holds tile t's 128 slots
        dl2 = dloc.reshape(t_pad, TILE)
        dl = dl2.T.astype(ml_dtypes.bfloat16)
        # compact per-tile selectors: b16 [dst->slot], sel32T [slot->edge];
        # 3 tiles share a 128-partition window (32 slots at bases 0/32/64)
        GRP = c_.GRP
        nwin = -(-GRP // 3)
        ng = t_pad // GRP
        b16 = np.zeros((TILE, ng * nwin * TILE), ml_dtypes.bfloat16)
        selt = np.zeros((TILE, ng * nwin * TILE), ml_dtypes.bfloat16)
        for t in range(t_pad):
            dd = dl2[t]
            m = dd >= 0
            if not m.any():
                continue
            u, inv = np.unique(dd[m], return_inverse=True)
            assert len(u) <= KSL, f"tile {t}: {len(u)} distinct dsts"
            g, tl = t // GRP, t % GRP
            blk = g * nwin + tl // 3
            a = tl % 3
            b16[u, blk * TILE + a * KSL + np.arange(len(u))] = 1
            ee = np.nonzero(m)[0]
            selt[a * KSL + inv, blk * TILE + ee] = 1
        b01 = np.zeros((TILE, c_.CH * c_.NGR), ml_dtypes.bfloat16)
        ii = np.arange(c_.NLOC)
        b01[ii % TILE, (ii // TILE) * c_.NGR + batch[k * c_.NLOC + ii]] = 1
        per_core.append({"gsrc1": wrap_idx(gi_src), "gsrc2": wrap_idx(gi_src2),
                         "dloc": dl, "b01": b01, "b16": b16, "selt": selt,
                         "_gi": gi_src})
    # per-gather-call ext1 row prefix (max over cores, 128-rounded): the
    # gather's table AP is sliced to this so it only depends on the ext1
    # writes it actually needs
    GH = 8
    ncall = t_pad // GH
    g1rows = np.zeros(ncall, np.int64)
    for pc in per_core:
        gi = pc.pop("_gi").reshape(ncall, GH * TILE)
        g1rows = np.maximum(g1rows, gi.max(axis=1) + 1)
    g1rows = np.minimum(-(-g1rows // TILE) * TILE, c_.NT1 * TILE)
    return cs, t_pad, per_core, g1rows


def make_in_maps(inputs, cfg, per_core, t_pad):
    import ml_dtypes
    c_ = cfg
    x = np.asarray(inputs["x"], np.float32)
    nt1 = c_.NT1
    x_pad = np.zeros((nt1 * TILE, c_.FIN), np.float32)
    x_pad[:c_.N] = x
    xT = np.ascontiguousarray(x_pad.T).astype(ml_dtypes.bfloat16)

    W1 = np.asarray(inputs["W1"], np.float32)
    as1 = np.asarray(inputs["att_src1"], np.float32)
    ad1 = np.asarray(inputs["att_dst1"], np.float32)
    W2 = np.asarray(inputs["W2"], np.float32)
    as2 = np.asarray(inputs["att_src2"], np.float32)
    ad2 = np.asarray(inputs["att_dst2"], np.float32)
    a1m = np.zeros((c_.D1, 2 * c_.H1), np.float32)
    for h in range(c_.H1):
        a1m[h * c_.C1:(h + 1) * c_.C1, h] = as1[h]
        a1m[h * c_.C1:(h + 1) * c_.C1, c_.H1 + h] = ad1[h]

    # permute conv2 feature space so argmax|att_src2| is feature 0, then
    # fold the a_src2 functional into that column of the conv2 weights
    # (M = h2p @ B2). The edge phase reads a_src2 as gathered col 0 and the
    # chunk epilogue inverts the fold.
    k2 = int(np.argmax(np.abs(as2[0])))
    perm2 = np.concatenate([[k2], np.delete(np.arange(c_.D2), k2)])
    W2p = W2[perm2, :]
    as2p = as2[0][perm2].astype(np.float32)
    ad2p = ad2[0][perm2].astype(np.float32)
    B2 = np.eye(c_.D2, dtype=np.float32)
    B2[:, 0] = as2p
    a2m = np.stack([as2p, ad2p], axis=1).astype(np.float32)
    as2mod = as2p.copy()
    as2mod[0] = 0.0

    rep = lambda v, w: np.tile(
        np.asarray(v, np.float32).reshape(1, w), (TILE, 1))
    w1tb = np.ascontiguousarray(W1.T).astype(ml_dtypes.bfloat16)
    wpa = W2p.copy()
    wpb = np.concatenate([W1, np.ascontiguousarray(W2p.T @ B2)], axis=0)
    apk = np.zeros((c_.D1, 2 * c_.H1 + 4), np.float32)
    apk[:, 0:2 * c_.H1] = a1m
    apk[0:c_.D2, 2 * c_.H1:2 * c_.H1 + 2] = a2m
    apk[0:c_.D2, 2 * c_.H1 + 2:2 * c_.H1 + 4] = np.ascontiguousarray(
        np.asarray(inputs["W_out"], np.float32)[:, perm2].T)
    smalls = np.concatenate([
        rep(np.asarray(inputs["b1"], np.float32), c_.D1),
        rep(np.asarray(inputs["b2"], np.float32)[perm2], c_.D2),
        rep(np.asarray(inputs["w_attn"], np.float32)[perm2, 0], c_.D2),
        rep(np.asarray(inputs["w_mask"], np.float32)[perm2, 0], c_.D2),
        rep(as2mod, c_.D2),
        np.full((TILE, 1), 1.0 / as2p[0], np.float32),
        np.eye(TILE, dtype=np.float32),
        rep(inputs["b_out"], c_.OUT),
        rep(inputs["b_attn"], 1),
        np.full((TILE, 1), 1e-16, np.float32),
        rep(inputs["b_mask"], 1),
    ], axis=1)
    iotaF = np.tile(np.arange(TILE, dtype=np.float32).reshape(1, TILE),
                    (TILE, 1)).astype(ml_dtypes.bfloat16)
    base = {
        "x": xT, "w1tb": w1tb, "wpa": wpa, "wpb": wpb, "apk": apk,
        "smalls": smalls, "tok": np.zeros((TILE, 8), np.float32),
    }
    in_maps = []
    for k in range(c_.NC):
        m = dict(base)
        pc = per_core[k]
        m["gidx"] = np.concatenate([pc["gsrc1"], pc["gsrc2"]], axis=1)
        m["dlp"] = np.concatenate([pc["dloc"], iotaF], axis=1)
        m["selp"] = pc["b01"]
        m["selb"] = pc["b16"]
        m["selt"] = pc["selt"]
        xloc = np.zeros((c_.FIN, c_.CH * TILE), ml_dtypes.bfloat16)
        xloc[:, 0:c_.NLOC] = xT[:, k * c_.NLOC:(k + 1) * c_.NLOC]
        m["xloc"] = xloc
        in_maps.append(m)
    return in_maps


_CACHE = {}


def run(inputs, cfg):
    from concourse.bass_utils import run_bass_kernel_spmd
    cs, t_pad, per_core, g1rows = host_prep(inputs, cfg)
    key = (cfg.N, t_pad, tuple(cs), tuple(g1rows))
    if key not in _CACHE:
        _CACHE[key] = build_program(cfg, cs, t_pad, g1rows)
    nc = _CACHE[key]
    in_maps = make_in_maps(inputs, cfg, per_core, t_pad)
    res = run_bass_kernel_spmd(nc, in_maps, list(range(cfg.NC)), trace=False)
    return np.asarray(res.results[0]["out"], np.float32)


def kernel(**inputs):
    return run(inputs, Cfg())


def _exec_maker(nc, in_maps, n_cores):
    """Build a jitted executor (structure identical to bass2jax's _body) and
    device-resident inputs. Returns (f, dev_args)."""
    import jax
    from jax.sharding import Mesh, PartitionSpec, NamedSharding
    from jax.experimental.shard_map import shard_map
    from concourse import mybir as mb
    from concourse.bass2jax import _bass_exec_p, partition_id_tensor, \
        install_neuronx_cc_hook

    install_neuronx_cc_hook()
    partition_name = (nc.partition_id_tensor.name
                      if nc.partition_id_tensor else None)
    in_names, out_names, out_avals, zero_outs = [], [], [], []
    for alloc in nc.m.functions[0].allocations:
        if not isinstance(alloc, mb.MemoryLocationSet):
            continue
        name = alloc.memorylocations[0].name
        if alloc.kind == "ExternalInput":
            if name != partition_name:
                in_names.append(name)
        elif alloc.kind == "ExternalOutput":
            out_names.append(name)
            shape = tuple(alloc.tensor_shape)
            dtype = mb.dt.np(alloc.dtype)
            out_avals.append(jax.core.ShapedArray(shape, dtype))
            zero_outs.append(np.zeros(shape, dtype))
    n_params = len(in_names)
    all_in = in_names + out_names
    if partition_name is not None:
        all_in = all_in + [partition_name]

    def _body(*args):
        ops = list(args)
        if partition_name is not None:
            ops.append(partition_id_tensor())
        outs = _bass_exec_p.bind(
            *ops, out_avals=tuple(out_avals), in_names=tuple(all_in),
            out_names=tuple(out_names), lowering_input_output_aliases=(),
            sim_require_finite=True, sim_require_nnan=True, nc=nc)
        return tuple(outs)

    devices = jax.devices()[:n_cores]
    mesh = Mesh(np.asarray(devices), ("core",))
    nin = n_params + len(zero_outs)
    f = jax.jit(shard_map(
        _body, mesh=mesh, in_specs=(PartitionSpec("core"),) * nin,
        out_specs=(PartitionSpec("core"),) * len(out_names),
        check_rep=False), keep_unused=True)
    per_core = [[np.asarray(in_maps[c][n]) for n in in_names] + zero_outs
                for c in range(n_cores)]
    concat_in = [np.concatenate([per_core[c][i] for c in range(n_cores)],
                                axis=0) for i in range(nin)]
    sh = NamedSharding(mesh, PartitionSpec("core"))
    dev_args = [jax.device_put(a, sh) for a in concat_in]
    return f, dev_args


def _build_tiny(n_cores):
    nc = bacc.Bacc("TRN2", target_bir_lowering=False, debug=False,
                   num_devices=n_cores)
    tok = nc.declare_dram_parameter("tok", [TILE, 8], F32, isOutput=False)
    tok_out = nc.declare_dram_parameter("tok_out", [TILE, 8], F32,
                                        isOutput=True)
    with tile.TileContext(nc) as tc, ExitStack() as ctx:
        pool = ctx.enter_context(tc.tile_pool(name="p", bufs=1))
        t = pool.tile([TILE, 8], F32)
        nc.sync.dma_start(t[:], tok[:])
        nc.sync.dma_start(tok_out[:], t[:])
    nc.compile()
    return nc


def _timed_slope(f, dev_args, reps=9, k_lo=2, k_hi=10):
    """Per-exec device time via slope fitting: wall time of k_hi chained
    executions minus wall time of k_lo, divided by (k_hi - k_lo). Chained
    async dispatches execute back-to-back on-device, so the tunnel
    round-trip and dispatch overhead cancel in the difference."""
    import jax
    import time as _t

    def run_k(k):
        outs = None
        t0 = _t.perf_counter()
        for _ in range(k):
            outs = f(*dev_args)
        jax.block_until_ready(outs)
        return _t.perf_counter() - t0

    run_k(2)
    run_k(2)
    lo, hi = [], []
    for _ in range(reps):
        lo.append(run_k(k_lo))
        hi.append(run_k(k_hi))
    lo.sort(); hi.sort()
    med_lo = lo[len(lo) // 2]
    med_hi = hi[len(hi) // 2]
    return (med_hi - med_lo) / (k_hi - k_lo), med_lo, med_hi


def measure_hw_time(inputs, reps=30, cfg=None, stage=99):
    """Per-execution device time estimate: wall time of the kernel with
    device-resident inputs, minus the same measurement for a trivial
    pass-through program (dispatch/tunnel baseline)."""
    cfg = cfg or Cfg()
    cs, t_pad, per_core, g1rows = host_prep(inputs, cfg)
    key = (cfg.N, t_pad, tuple(cs), tuple(g1rows), stage)
    if key not in _CACHE:
        _CACHE[key] = build_program(cfg, cs, t_pad, g1rows, stage=stage)
    nc = _CACHE[key]
    in_maps = make_in_maps(inputs, cfg, per_core, t_pad)
    f, dev_args = _exec_maker(nc, in_maps, cfg.NC)
    per_exec, med_lo, med_hi = _timed_slope(f, dev_args)
    print(f"slope fit: k2={med_lo*1e3:.2f}ms k10={med_hi*1e3:.2f}ms "
          f"-> per-exec {per_exec*1e3:.3f} ms")
    return per_exec * 1e9
